# revision 4
# baseline (speedup 1.0000x reference)
"""CrossEntropyLoss kernel for Trainium2, SPMD over 8 NeuronCores.

reference:
    gathered = output[i, label[i]]                      # [B]
    loss = (sum_i -gathered_i + sum_i log(sum_j exp(output[i,j]) + 1e-5)) / B

Sharding: batch (B=8192) split across 8 cores, 1024 rows per core.

The kernel is HBM-bandwidth-bound (DMA-only streaming of the f32 shard
measures ~290us, the same as the full kernel), so the host casts the
shard to bf16 before upload: 65.5 MB instead of 131 MB per core. The
rel-err budget (2e-2) dwarfs the bf16 quantization error (~1e-4 on the
loss). Per core: stream the [1024, 32000] bf16 shard in [128, 16000]
chunks; ACT computes exp with fused row-sum accumulation (accum_out);
label gather is one indirect DMA per row tile; ln(sumexp + eps) -
gathered per row returns to the host, which sums and divides by B.
"""

import numpy as np

import concourse.bass as bass
import concourse.mybir as mybir
import concourse.tile as tile
from concourse.bass_utils import run_bass_kernel_spmd

B, V = 8192, 32000
N_CORES = 8
B_LOC = B // N_CORES  # 1024 rows per core
P = 128  # SBUF partitions
EPS = 1e-5


def split_multi_waits(nc):
    """This walrus build's CoreV2/V3 codegen rejects any instruction carrying
    more than one sync wait command. Split extra waits onto same-engine NoOps
    inserted immediately before the offending instruction (sequential waits on
    one engine are equivalent to one AND-ed wait set)."""
    n_split = 0
    for func in nc.m.functions:
        for block in func.blocks:
            new_insts = []
            for inst in block.instructions:
                si = inst.sync_info
                if si is not None and len(si.on_wait) > 1:
                    waits = list(si.on_wait)
                    for w in waits[:-1]:
                        nop = mybir.InstNoOp(
                            name=f"I-waitsplit-{nc.next_id()}",
                            sync_info=mybir.SyncInfo(on_wait=[w], on_update=[]),
                            bass_nofuse=True,
                            engine=inst.engine,
                        )
                        nc.register_instruction(nop)
                        new_insts.append(nop)
                        n_split += 1
                    si.on_wait = [waits[-1]]
                new_insts.append(inst)
            block.instructions[:] = new_insts
    return n_split


LOG2E = 1.4426950408889634
SCHRA_A = 128.0 * LOG2E            # bf16-bit-domain scale
SCHRA_B = 127.0 * 128.0 - 7.33     # exponent bias minus mean-zero correction


def build_nc(b_loc=B_LOC, v=V, dma_chunk=16000, act_chunk=4000, dve_cols=4800,
             xin_bufs=3, repeat=1):
    """Build the single-core Bass program (same program runs SPMD on all cores).

    Per DMA chunk of `dma_chunk` bf16 columns, the last `dve_cols` columns are
    offloaded to the vector engine (Schraudolph exp: int16(x*A+B) bitcast to
    bf16, then a pair-add scalar_tensor_tensor with fused row-sum), the rest
    go through ACT exp with fused accumulation. Both engines run concurrently;
    dve_cols balances their per-chunk times.

    repeat>1 re-runs the streaming phase (identical work/results) so one
    dispatch holds R x the device work - used only for timing measurements.
    """
    assert b_loc % P == 0 and v % dma_chunk == 0
    assert dve_cols % 2 == 0
    act_cols = dma_chunk - dve_cols
    n_rt = b_loc // P  # row tiles of 128 rows
    n_dc = v // dma_chunk  # DMA chunks per row tile
    # ACT sub-chunks (PSUM free-dim cap is 4K)
    act_splits = []
    o = 0
    while o < act_cols:
        w = min(act_chunk, act_cols - o)
        act_splits.append((o, w))
        o += w
    cpr = (len(act_splits) + (1 if dve_cols else 0)) * n_dc  # partials cols per row tile

    nc = bass.Bass()
    x = nc.dram_tensor("x", [b_loc, v], mybir.dt.bfloat16, kind="ExternalInput")
    idx = nc.dram_tensor("idx", [P, n_rt], mybir.dt.int32, kind="ExternalInput")
    out = nc.dram_tensor("out", [P, n_rt], mybir.dt.float32, kind="ExternalOutput")

    x_flat = x[:].rearrange("a (b one) -> (a b) one", one=1)

    with tile.TileContext(nc) as tc:
        with (
            tc.tile_pool(name="xin", bufs=xin_bufs) as xin,
            tc.tile_pool(name="ebuf", bufs=2) as ebuf,
            tc.tile_pool(name="trash", bufs=1, space="PSUM") as trash,
            tc.tile_pool(name="small", bufs=1) as small,
        ):
            # Label gather: overlaps with the streaming loop (reads DRAM only).
            idx_t = small.tile([P, n_rt], mybir.dt.int32)
            nc.sync.dma_start(out=idx_t[:], in_=idx[:])
            g_t = small.tile([P, n_rt], mybir.dt.bfloat16)
            # One [128,1] gather per row tile: multi-column offset APs
            # mis-address on HW (verified), per-column gathers are exact.
            for rt in range(n_rt):
                nc.gpsimd.indirect_dma_start(
                    out=g_t[:, rt : rt + 1],
                    out_offset=None,
                    in_=x_flat,
                    in_offset=bass.IndirectOffsetOnAxis(
                        ap=idx_t[:, rt : rt + 1], axis=0
                    ),
                )

            # Per-engine partials tiles (separate so ACT and DVE instruction
            # streams share no written tile and never cross-serialize).
            nsa = len(act_splits)
            partials_a = small.tile([P, n_rt * n_dc * nsa], mybir.dt.float32)
            if dve_cols:
                partials_d = small.tile([P, n_rt * n_dc], mybir.dt.float32)
            for _rep in range(repeat):
              for rt in range(n_rt):
                for dc in range(n_dc):
                    x_t = xin.tile([P, dma_chunk], mybir.dt.bfloat16, tag="x")
                    nc.sync.dma_start(
                        out=x_t[:],
                        in_=x[rt * P : (rt + 1) * P, dc * dma_chunk : (dc + 1) * dma_chunk],
                    )
                    ci = rt * n_dc + dc
                    for si, (o, w) in enumerate(act_splits):
                        e_t = trash.tile([P, act_chunk], mybir.dt.float32, tag="e")
                        nc.scalar.activation(
                            out=e_t[:, 0:w],
                            in_=x_t[:, o : o + w],
                            func=mybir.ActivationFunctionType.Exp,
                            accum_out=partials_a[:, ci * nsa + si : ci * nsa + si + 1],
                        )
                    if dve_cols:
                        y_t = ebuf.tile([P, dve_cols], mybir.dt.int16, tag="y")
                        nc.vector.tensor_scalar(
                            out=y_t[:],
                            in0=x_t[:, act_cols:dma_chunk],
                            scalar1=SCHRA_A, scalar2=SCHRA_B,
                            op0=mybir.AluOpType.mult, op1=mybir.AluOpType.add,
                        )
                        e_bf = y_t[:].bitcast(mybir.dt.bfloat16)
                        h = dve_cols // 2
                        pair_t = ebuf.tile([P, h], mybir.dt.bfloat16, tag="p")
                        nc.vector.scalar_tensor_tensor(
                            out=pair_t[:],
                            in0=e_bf[:, 0:h], scalar=1.0, in1=e_bf[:, h : 2 * h],
                            op0=mybir.AluOpType.mult, op1=mybir.AluOpType.add,
                            accum_out=partials_d[:, ci : ci + 1],
                        )

            # Combine: sumexp per row -> ln(. + eps) -> minus gathered logit.
            sums = small.tile([P, n_rt], mybir.dt.float32)
            for rt in range(n_rt):
                nc.vector.reduce_sum(
                    out=sums[:, rt : rt + 1],
                    in_=partials_a[:, rt * n_dc * nsa : (rt + 1) * n_dc * nsa],
                    axis=mybir.AxisListType.X,
                )
            if dve_cols:
                sums_d = small.tile([P, n_rt], mybir.dt.float32)
                for rt in range(n_rt):
                    nc.vector.reduce_sum(
                        out=sums_d[:, rt : rt + 1],
                        in_=partials_d[:, rt * n_dc : (rt + 1) * n_dc],
                        axis=mybir.AxisListType.X,
                    )
                nc.vector.tensor_add(out=sums[:], in0=sums[:], in1=sums_d[:])
            eps_t = small.tile([P, 1], mybir.dt.float32)
            nc.gpsimd.memset(eps_t[:], EPS)
            lg_t = small.tile([P, n_rt], mybir.dt.float32)
            nc.scalar.activation(
                out=lg_t[:],
                in_=sums[:],
                func=mybir.ActivationFunctionType.Ln,
                bias=eps_t[:],
            )
            g32 = small.tile([P, n_rt], mybir.dt.float32)
            nc.vector.tensor_copy(out=g32[:], in_=g_t[:])
            res_t = small.tile([P, n_rt], mybir.dt.float32)
            nc.vector.tensor_sub(out=res_t[:], in0=lg_t[:], in1=g32[:])
            nc.sync.dma_start(out=out[:], in_=res_t[:])

    split_multi_waits(nc)
    return nc


def make_in_maps(output, label, b_loc=B_LOC, v=V, n_cores=N_CORES):
    """Shard full inputs into per-core input maps (x cast to bf16 host-side)."""
    import ml_dtypes

    output = np.asarray(output)
    label = np.asarray(label).astype(np.int64)
    n_rt = b_loc // P
    in_maps = []
    for c in range(n_cores):
        xs = np.ascontiguousarray(
            output[c * b_loc : (c + 1) * b_loc].astype(ml_dtypes.bfloat16)
        )
        ls = label[c * b_loc : (c + 1) * b_loc]
        flat = (np.arange(b_loc, dtype=np.int64) * v + ls).astype(np.int32)
        idx_mat = np.ascontiguousarray(flat.reshape(n_rt, P).T)  # [p, rt]
        in_maps.append({"x": xs, "idx": idx_mat})
    return in_maps


def combine(results, b=B):
    """Sum per-row terms from all cores and divide by the batch size."""
    total = 0.0
    for r in results:
        total += r["out"].astype(np.float64).sum()
    return np.float32(total / b)


_NC_CACHE = {}


def kernel(output, label):
    if "nc" not in _NC_CACHE:
        _NC_CACHE["nc"] = build_nc()
    nc = _NC_CACHE["nc"]
    in_maps = make_in_maps(output, label)
    res = run_bass_kernel_spmd(nc, in_maps, list(range(N_CORES)))
    return combine(res.results)
